# revision 2
# baseline (speedup 1.0000x reference)
"""AlignBlock Bass kernel for 8 trn2 NeuronCores.

Sharding: pure data-parallel over (batch b=2) x (time chunks of 128) = 8 shards.
Each core computes out[bi, :, t0:t0+128, :] from haloed input slices.

Per-core device algorithm (all fp32):
  1. xm1/xf1 projections c->h done as per-timechunk matmuls producing the
     TRANSPOSED layout X2[(frame-parity, f), (k, h)] directly (lhsT = xm chunk).
     Bias added via pre-broadcast [128,h] tensors; out-of-range halo frames
     zeroed with host-sent per-partition masks (tensor_scalar_mul).
  2. Per-head correlation P_h = xm1_h @ xf1_h^T via 4 parity matmuls (K=f=64),
     merged in SBUF and stored contiguously to a DRAM scratch P[h, i, j]
     (row stride 164 so the delay band P[h,i,i+e] is an affine AP, stride 165).
  3. Band+conv: gather G3[(dshift,h), i, d] = P[h, i, i+d+dshift-1] with one
     affine DMA, zero the two invalid d-edge strips, then ONE K=96 matmul
     with lhsT=wc arranged [(dshift,h), kt] -> out5[kt, i, d]. The kt (time)
     shifts are summed by writing out5 to DRAM and reading back a shifted
     view B5[t, kt, d] (affine), then 4 vector adds.
  4. mask+bias add (host-sent, includes conv bias bc and the causal -1e13),
     softmax over d on [t=128, 32].
  5. A@V: scatter softmax weights into a zeroed banded matrix Wt[u, t] in DRAM
     (affine AP), read back as [u,t] lhsT chunks, and matmul against
     host-pretransposed xf in [u, (c,f)] layout, accumulating over 2 K-chunks.
     Output lands directly as OUT[t, (c,f)]; host transposes to [c, t, f].
"""

import numpy as np

C, T, F, H, D = 128, 512, 64, 32, 32
TLOC = 128            # time frames per core
NXM = TLOC + 4        # xm1 frames: [t0-3, t0+129)  -> 132
NXF = TLOC + 36       # xf frames:  [t0-34, t0+130) -> 164 (even, last frame maybe pad)
KXM = NXM // 2        # 66 parity chunks
KXF = NXF // 2        # 82
PR = NXF              # P row stride (j dim padded to 164)
PH = NXM * PR         # per-head P size 132*164 = 21648
NEG = -1e13

_CACHE = {}


def _build_program():
    import concourse.bass as bass
    import concourse.mybir as mybir
    from concourse import bacc, tile

    fp32 = mybir.dt.float32
    nc = bacc.Bacc("TRN2", target_bir_lowering=False, debug=False)

    XM = nc.dram_tensor("XM", [C, NXM * F], fp32, kind="ExternalInput")
    XF2 = nc.dram_tensor("XF2", [C, NXF * F], fp32, kind="ExternalInput")
    XFTa = nc.dram_tensor("XFTa", [128, C * F], fp32, kind="ExternalInput")
    XFTb = nc.dram_tensor("XFTb", [NXF - 128, C * F], fp32, kind="ExternalInput")
    W1 = nc.dram_tensor("W1", [C, H], fp32, kind="ExternalInput")
    W2 = nc.dram_tensor("W2", [C, H], fp32, kind="ExternalInput")
    B1 = nc.dram_tensor("B1", [128, H], fp32, kind="ExternalInput")
    B2 = nc.dram_tensor("B2", [128, H], fp32, kind="ExternalInput")
    WCK = nc.dram_tensor("WCK", [96, 5], fp32, kind="ExternalInput")
    MXM = nc.dram_tensor("MXM", [64, 2 * KXM], fp32, kind="ExternalInput")
    MXF = nc.dram_tensor("MXF", [64, 2 * KXF], fp32, kind="ExternalInput")
    MCORR = nc.dram_tensor("MCORR", [TLOC, D], fp32, kind="ExternalInput")
    WTZ = nc.dram_tensor("WTZ", [128, NXF], fp32, kind="ExternalInput")
    OUT = nc.dram_tensor("OUT", [TLOC, C * F], fp32, kind="ExternalOutput")

    with tile.TileContext(nc) as tc:
        with (
            tc.tile_pool(name="big", bufs=1) as big,
            tc.tile_pool(name="stream", bufs=4) as stream,
            tc.tile_pool(name="small", bufs=2) as small,
            tc.tile_pool(name="s2p", bufs=4) as s2p,
            tc.tile_pool(name="psum", bufs=6, space="PSUM") as psum,
            tc.tile_pool(name="dram", bufs=1, space="DRAM") as dram,
        ):
            # ---- weights / masks in SBUF
            w1_sb = big.tile([C, H], fp32, tag="w1")
            w2_sb = big.tile([C, H], fp32, tag="w2")
            b1_sb = big.tile([128, H], fp32, tag="b1")
            b2_sb = big.tile([128, H], fp32, tag="b2")
            wck_sb = big.tile([96, 5], fp32, tag="wck")
            mxm_sb = big.tile([64, 2 * KXM], fp32, tag="mxm")
            mxf_sb = big.tile([64, 2 * KXF], fp32, tag="mxf")
            mcorr_sb = big.tile([TLOC, D], fp32, tag="mcorr")
            for sb, dr in ((w1_sb, W1), (w2_sb, W2), (b1_sb, B1), (b2_sb, B2),
                           (wck_sb, WCK), (mxm_sb, MXM), (mxf_sb, MXF),
                           (mcorr_sb, MCORR)):
                nc.sync.dma_start(sb[:], dr[:])

            xm_sb = big.tile([C, NXM * F], fp32, tag="xmsb")
            xf2_sb = big.tile([C, NXF * F], fp32, tag="xf2sb")
            nc.sync.dma_start(xm_sb[:], XM[:])
            nc.sync.dma_start(xf2_sb[:], XF2[:])

            # zero the banded-weight DRAM scratch early (no deps -> few waits)
            Wtd = dram.tile([NXF * TLOC], fp32, tag="wtd")
            wt_ap = Wtd[:]
            wt_t, wt_off = wt_ap.tensor, int(wt_ap.offset)
            nc.sync.dma_start(bass.AP(wt_t, wt_off, [[1, NXF * TLOC]]), WTZ[:])

            # ---- projections -> X2mE/X2mO [f=64, k, h] (even/odd frames)
            X2mE = big.tile([64, KXM, H], fp32, tag="x2me")
            X2mO = big.tile([64, KXM, H], fp32, tag="x2mo")
            X2fE = big.tile([64, KXF, H], fp32, tag="x2fe")
            X2fO = big.tile([64, KXF, H], fp32, tag="x2fo")

            def project(nk, src_sb, w_sb, b_sb, m_sb, dstE, dstO):
                for k in range(nk):
                    for par, dst in ((0, dstE), (1, dstO)):
                        chunk = src_sb[:, k * 128 + par * 64:k * 128 + par * 64 + 64]
                        ps = psum.tile([64, H], fp32, tag="ps")
                        nc.tensor.matmul(ps[:], chunk, w_sb[:], start=True, stop=True)
                        nc.vector.tensor_add(dst[:, k, :], ps[:], b_sb[0:64, :])
                for par, dst in ((0, dstE), (1, dstO)):
                    mb = m_sb[:, par * nk:(par + 1) * nk].unsqueeze(2)
                    mb = mb.broadcast_to([64, nk, H])
                    nc.vector.tensor_mul(dst[:], dst[:], mb)

            project(KXM, xm_sb, w1_sb, b1_sb, mxm_sb, X2mE, X2mO)
            project(KXF, xf2_sb, w2_sb, b2_sb, mxf_sb, X2fE, X2fO)

            # ---- per-head correlation, parity matmuls, merge, store to P
            Pd = dram.tile([H * PH + 2], fp32, tag="pd")
            pd_ap = Pd[:]
            pd_t, pd_off = pd_ap.tensor, int(pd_ap.offset)

            for h in range(H):
                psq = {}
                for di, xm2 in ((0, X2mE), (1, X2mO)):
                    lhsT = xm2[:, :, h]                      # [64, 66]
                    for dj, xf2p in ((0, X2fE), (1, X2fO)):
                        rhs = xf2p[:, :, h]                  # [64, 82]
                        ps = psum.tile([KXM, KXF], fp32, tag="ps")
                        nc.tensor.matmul(ps[:], lhsT, rhs, start=True, stop=True)
                        psq[(di, dj)] = ps
                s2 = s2p.tile([KXM, 2, KXF, 2], fp32, tag="s2")
                for di in range(2):
                    for dj in range(2):
                        nc.vector.tensor_copy(s2[:, di, :, dj], psq[(di, dj)][:])
                dst = bass.AP(pd_t, pd_off + 1 + h * PH,
                              [[2 * KXF * 2, KXM], [164, 2], [1, 164]])
                nc.sync.dma_start(dst, s2[:])

            # ---- band gather + 5x3 conv (contract (dshift,h)=96 on PE)
            G3 = big.tile([96, NXM, D], fp32, tag="g3")
            for b in range(3):
                src = bass.AP(pd_t, pd_off + b,
                              [[PH, H], [PR + 1, NXM], [1, D]])
                nc.sync.dma_start(G3[b * 32:(b + 1) * 32, :, :], src)
            nc.vector.memset(G3[0:32, :, 0], 0.0)
            nc.vector.memset(G3[64:96, :, D - 1], 0.0)

            O5 = big.tile([5, NXM * D], fp32, tag="o5")
            g3f = G3[:].rearrange("p a b -> p (a b)")
            ncols = NXM * D  # 4224
            nch = (ncols + 511) // 512
            for n in range(nch):
                sl = slice(n * 512, min(ncols, (n + 1) * 512))
                ps5 = psum.tile([5, 512], fp32, tag="ps")
                w = sl.stop - sl.start
                nc.tensor.matmul(ps5[:, :w], wck_sb[:], g3f[:, sl], start=True, stop=True)
                nc.vector.tensor_copy(O5[:, sl], ps5[:, :w])

            O5d = dram.tile([5 * NXM * D], fp32, tag="o5d")
            o5_ap = O5d[:]
            o5_t, o5_off = o5_ap.tensor, int(o5_ap.offset)
            nc.sync.dma_start(bass.AP(o5_t, o5_off, [[NXM * D, 5], [64, 66], [1, 64]]),
                              O5[:].rearrange("p (a b) -> p a b", a=66))

            B5 = small.tile([TLOC, 5, D], fp32, tag="b5")
            bsrc = bass.AP(o5_t, o5_off, [[D, TLOC], [NXM * D + D, 5], [1, D]])
            nc.sync.dma_start(B5[:], bsrc)

            acc = small.tile([TLOC, D], fp32, tag="acc")
            nc.vector.tensor_add(acc[:], B5[:, 0, :], B5[:, 1, :])
            for a in range(2, 5):
                nc.vector.tensor_add(acc[:], acc[:], B5[:, a, :])
            sm = small.tile([TLOC, D], fp32, tag="sm")
            nc.vector.tensor_add(sm[:], acc[:], mcorr_sb[:])

            # ---- softmax over d
            mx = small.tile([TLOC, 1], fp32, tag="mx")
            nc.vector.reduce_max(mx[:], sm[:], axis=mybir.AxisListType.X)
            nmx = small.tile([TLOC, 1], fp32, tag="nmx")
            nc.vector.tensor_scalar_mul(nmx[:], mx[:], -1.0)
            ex = small.tile([TLOC, D], fp32, tag="ex")
            nc.scalar.activation(ex[:], sm[:], mybir.ActivationFunctionType.Exp,
                                 bias=nmx[:])
            ssum = small.tile([TLOC, 1], fp32, tag="ssum")
            nc.vector.reduce_sum(ssum[:], ex[:], axis=mybir.AxisListType.X)
            rs = small.tile([TLOC, 1], fp32, tag="rs")
            nc.vector.reciprocal(rs[:], ssum[:])
            wgt = small.tile([TLOC, D], fp32, tag="wgt")
            nc.vector.tensor_scalar_mul(wgt[:], ex[:], rs[:])

            # ---- scatter weights into banded Wt[u, t] (u = t + 3 + d)
            sct = bass.AP(wt_t, wt_off + 3 * TLOC,
                          [[TLOC + 1, TLOC], [TLOC * 8, 4], [TLOC, 8]])
            nc.sync.dma_start(sct, wgt[:].rearrange("p (a b) -> p a b", a=4))

            wta = small.tile([128, TLOC], fp32, tag="wta")
            wtb = small.tile([NXF - 128, TLOC], fp32, tag="wtb")
            nc.sync.dma_start(wta[:].rearrange("p (a b) -> p a b", a=2),
                              bass.AP(wt_t, wt_off, [[TLOC, 128], [64, 2], [1, 64]]))
            nc.sync.dma_start(wtb[:].rearrange("p (a b) -> p a b", a=2),
                              bass.AP(wt_t, wt_off + 128 * TLOC,
                                      [[TLOC, NXF - 128], [64, 2], [1, 64]]))

            # ---- A @ V: out[t, (c,f)] = sum_u Wt[u,t] * XFT[u, (c,f)]
            for n in range(16):
                sl = slice(n * 512, (n + 1) * 512)
                va = stream.tile([128, 512], fp32, tag="va")
                vb = stream.tile([NXF - 128, 512], fp32, tag="vb")
                nc.sync.dma_start(va[:], XFTa[:, sl])
                nc.sync.dma_start(vb[:], XFTb[:, sl])
                ps = psum.tile([TLOC, 512], fp32, tag="ps")
                nc.tensor.matmul(ps[:], wta[:], va[:], start=True, stop=False)
                nc.tensor.matmul(ps[:], wtb[:], vb[:], start=False, stop=True)
                ob = stream.tile([TLOC, 512], fp32, tag="ob")
                nc.vector.tensor_copy(ob[:], ps[:])
                nc.sync.dma_start(OUT[:, sl], ob[:])

    nc.compile()
    return nc


def _prep_core(bi, ch, xm, xf, bc):
    t0 = ch * TLOC

    def tslice(x, lo, n):
        out = np.zeros((C, n, F), dtype=np.float32)
        glo, ghi = max(0, lo), min(T, lo + n)
        if ghi > glo:
            out[:, glo - lo:ghi - lo, :] = x[:, glo:ghi, :]
        return out

    xm_s = tslice(xm[bi], t0 - 3, NXM)
    xf_s = tslice(xf[bi], t0 - 34, NXF)
    xft = np.ascontiguousarray(xf_s.transpose(1, 0, 2)).reshape(NXF, C * F)

    def mask(lo, nk):
        g = lo + np.concatenate([2 * np.arange(nk), 2 * np.arange(nk) + 1])
        v = ((g >= 0) & (g < T)).astype(np.float32)
        return np.tile(v[None, :], (64, 1))

    tloc = np.arange(TLOC)[:, None]
    dd = np.arange(D)[None, :]
    mcorr = np.where((t0 + tloc + dd) < (D - 1), NEG, 0.0).astype(np.float32) + bc[0]

    return {
        "XM": xm_s.reshape(C, NXM * F),
        "XF2": xf_s.reshape(C, NXF * F),
        "XFTa": xft[:128],
        "XFTb": xft[128:],
        "MXM": mask(t0 - 3, KXM),
        "MXF": mask(t0 - 34, KXF),
        "MCORR": mcorr.astype(np.float32),
        "WTZ": np.zeros((128, NXF), np.float32),
    }


def kernel(xm, xf, w1, b1, w2, b2, wc, bc):
    from concourse import bass_utils

    xm = np.asarray(xm, np.float32)
    xf = np.asarray(xf, np.float32)
    shared = {
        "W1": np.asarray(w1, np.float32),
        "W2": np.asarray(w2, np.float32),
        "B1": np.tile(np.asarray(b1, np.float32)[None, :], (128, 1)),
        "B2": np.tile(np.asarray(b2, np.float32)[None, :], (128, 1)),
        "WCK": np.ascontiguousarray(
            np.asarray(wc, np.float32)[0].transpose(2, 0, 1).reshape(96, 5)),
    }
    if "nc" not in _CACHE:
        _CACHE["nc"] = _build_program()
    nc = _CACHE["nc"]

    in_maps = []
    for core in range(8):
        bi, ch = core // 4, core % 4
        m = _prep_core(bi, ch, xm, xf, np.asarray(bc, np.float32))
        m.update(shared)
        in_maps.append(m)

    res = bass_utils.run_bass_kernel_spmd(nc, in_maps, core_ids=list(range(8)))
    _CACHE["last_res"] = res
    out = np.zeros((2, C, T, F), dtype=np.float32)
    for core in range(8):
        bi, ch = core // 4, core % 4
        o = res.results[core]["OUT"].reshape(TLOC, C, F).transpose(1, 0, 2)
        out[bi, :, ch * TLOC:(ch + 1) * TLOC, :] = o
    return out

